# revision 13
# baseline (speedup 1.0000x reference)
"""LogLeakLIF recurrent SNN kernel for Trainium2 (8 NeuronCores, batch-sharded).

Math (validated vs reference, 0 spike flips over full T=1000):
  h == 1 always (i_in never exactly 0), so the t-state drops out; the
  log2 leak is near-total (r(u) = u - log2(2^u + 1e-5) ~ -1.44e-5 * 2^-u),
  so the recurrence collapses to
      v(t) = i_x(t) + z(t-1) @ w_rec - copysign(|C0|, v(t-1))
      z(t) = (v(t) > 0.5)
  (the C1*v term of the deg-1 fit of r is ~1e-5 * v and dropping it was
  verified to produce 0 spike flips and v_maxabs ~3e-5 vs the reference).

Device layout per core (batch shard Bc=16): neuron-major tiles [128, 32],
partition p = n mod 128, column = (n // 128)*16 + b.  x is host-transposed to
xT[n, t*16+b] so i_x = x @ w_in becomes phase-A stationary-w_in matmuls that
run ahead of the serial recurrence; nixp stores 0.5 - i_x.

Per step the critical cycle is TensorE(4 matmuls z@w_rec) -> DVE
(z = is_gt(psum, thr)) -> TensorE; v/thr bookkeeping overlaps the next
step's matmuls.  thr(t) = copysign(|C0|, v(t)) + 0.5 - i_x(t+1).

build_program(T, reps=R) wraps the whole body in a hardware For_i loop:
one dispatch runs R identical passes, which lets test.py measure the real
per-pass device time by slope, cancelling the ~50-90ms axon dispatch floor.
"""
import os
import sys
import numpy as np

sys.path.insert(0, "/opt/trn_rl_repo")

import concourse.bass as bass
import concourse.bacc as bacc
import concourse.mybir as mybir
from concourse.tile import TileContext
from concourse.bass_utils import run_bass_kernel_spmd
from concourse.alu_op_type import AluOpType

F32 = mybir.dt.float32
F32R = mybir.dt.float32r
F16 = mybir.dt.float16
I32 = mybir.dt.int32

# deg-1 minimax fit of r(u) = u - log2(2^u + 1e-5) on [0,1]
C0 = -1.43969181e-05
SIGNMASK = -2 ** 31
ABS_C0_BITS = int(np.float32(-C0).view(np.int32))

N_CORES = 8
B_FULL = 128
BC = B_FULL // N_CORES          # 16 batch rows per core
N = 256                          # neurons (= n_in = n_rec)
COLS = 2 * BC                    # 32 free columns per step tile

_program_cache = {}


def build_program(T, reps=1, G=16, LA=64, NR=256, CH=128, RING=128,
                  DRAIN=32, PIECES_PER_STEP=2, XDT=F32, RDT=F16):
    """RDT is the recurrent-path dtype: w_rec, z0 and the z ring travel as
    fp16 (w_rec rounding verified to give 0 spike flips; z in {0,1} is
    exact in fp16). The host supplies w_rec/z0 pre-cast and upcasts z_out."""
    nc = bacc.Bacc()

    xt_d = nc.dram_tensor("xt", [N, T * BC], F32, kind="ExternalInput")
    win_d = nc.dram_tensor("w_in", [N, N], F32, kind="ExternalInput")
    wrec_d = nc.dram_tensor("w_rec", [N, N], RDT, kind="ExternalInput")
    z0_d = nc.dram_tensor("z0t", [128, COLS], RDT, kind="ExternalInput")
    v0_d = nc.dram_tensor("v0t", [128, COLS], F32, kind="ExternalInput")
    zout_d = nc.dram_tensor("z_out", [128, T * COLS], RDT,
                            kind="ExternalOutput")
    vout_d = nc.dram_tensor("v_out", [128, T * COLS], F32, kind="ExternalOutput")

    if T < NR:
        NR = max(G, T)
    if T < CH:
        CH = T
    if LA > T:
        LA = T
    # groups of G steps (tail group may be smaller); NR must be a multiple
    # of G so nixp-ring writes never straddle the ring boundary
    assert NR % G == 0
    group_starts = list(range(0, T, G))
    n_groups = len(group_starts)
    n_chunks = (T + CH - 1) // CH

    with TileContext(nc) as tc:
        with (
            tc.tile_pool(name="consts", bufs=1) as consts,
            tc.tile_pool(name="nixp_p", bufs=1) as nixp_p,
            tc.tile_pool(name="rings", bufs=1) as rings,
            tc.tile_pool(name="xtp", bufs=3) as xtp,
            tc.tile_pool(name="psumA", bufs=2, space="PSUM") as psumA,
            tc.tile_pool(name="psumB", bufs=4, space="PSUM") as psumB,
            tc.tile_pool(name="small", bufs=2) as small,
        ):
            import contextlib

            loop_ctx = tc.For_i(0, reps, 1) if reps > 1 else contextlib.nullcontext()
            with loop_ctx:
                # ---- constants ----
                win_t = [[consts.tile([128, 128], XDT, name=f"win{i}{j}",
                                      tag=f"win{i}{j}")
                          for j in range(2)] for i in range(2)]
                wrec_t = [[consts.tile([128, 128], RDT, name=f"wrec{i}{j}",
                                       tag=f"wrec{i}{j}")
                           for j in range(2)] for i in range(2)]
                for i in range(2):
                    for j in range(2):
                        nc.sync.dma_start(
                            out=win_t[i][j][:].bitcast(F32),
                            in_=win_d[i * 128:(i + 1) * 128, j * 128:(j + 1) * 128])
                        nc.sync.dma_start(
                            out=wrec_t[i][j][:],
                            in_=wrec_d[i * 128:(i + 1) * 128, j * 128:(j + 1) * 128])
                z0t = consts.tile([128, COLS], RDT, name="z0t", tag="z0t")
                v0t = consts.tile([128, COLS], F32, name="v0t", tag="v0t")
                nc.sync.dma_start(out=z0t[:], in_=z0_d[:, :])
                nc.sync.dma_start(out=v0t[:], in_=v0_d[:, :])
                halfc = consts.tile([128, 1], F32, name="halfc", tag="halfc")
                nc.gpsimd.memset(halfc[:], 0.5)

                # ---- nixp ring: 0.5 - i_x ----
                nixp = nixp_p.tile([128, NR * COLS], F32, name="nixp", tag="nixp")
                nixp3 = nixp[:].rearrange("p (t s) -> p t s", s=COLS)

                xh_tiles = {}

                def load_chunk(c):
                    if c >= n_chunks or c in xh_tiles:
                        return
                    ncols = min(CH * BC, T * BC - c * CH * BC)
                    xh = []
                    for i in range(2):
                        xi = xtp.tile([128, CH * BC], XDT, name=f"x{i}",
                                      tag=f"x{i}")
                        nc.sync.dma_start(
                            out=xi[:, :ncols].bitcast(F32),
                            in_=xt_d[i * 128:(i + 1) * 128,
                                     c * CH * BC:c * CH * BC + ncols])
                        xh.append(xi)
                    xh_tiles[c] = xh

                groupstate = {}

                def group_size(g):
                    t0 = group_starts[g]
                    return min(G, T - t0)

                def issue_piece(g, piece):
                    """Pieces 0..5 of phase-A group g: 0-3 matmul (j,i);
                    4,5: ACT copy per j half."""
                    t0 = group_starts[g]
                    gs = group_size(g)
                    c = t0 // CH
                    off = (t0 - c * CH) * BC
                    if piece == 0:
                        groupstate[g] = [
                            psumA.tile([128, G * BC], F32, name=f"pA{j}",
                                       tag=f"pA{j}")
                            for j in range(2)]
                    if piece < 4:
                        j, i = divmod(piece, 2)
                        xh = xh_tiles[c]
                        nc.tensor.matmul(
                            groupstate[g][j][:, :gs * BC],
                            win_t[i][j][:],
                            xh[i][:, off:off + gs * BC],
                            start=(i == 0), stop=(i == 1))
                    else:
                        j = piece - 4
                        rs = t0 % NR
                        dst = nixp3[:, rs:rs + gs, j * BC:(j + 1) * BC]
                        src = (groupstate[g][j][:, :gs * BC]
                               .rearrange("p (t s) -> p t s", s=BC))
                        nc.scalar.activation(
                            dst, src, mybir.ActivationFunctionType.Identity,
                            bias=halfc[:, 0:1], scale=-1.0)
                        if piece == 5:
                            del groupstate[g]

                # ---- output rings ----
                vring = rings.tile([128, RING * COLS], F32, name="vring",
                                   tag="vring")
                # zring carries the matmul moving dtype; layout-identical to
                # f32, DVE/DMA access it through a bitcast view
                zring = rings.tile([128, RING * COLS], RDT, name="zring",
                                   tag="zring")

                # ---- prologue: x chunks + phase A for [0, LA) ----
                load_chunk(0)
                load_chunk(1)
                g = 0
                while g < n_groups and group_starts[g] < LA:
                    for p in range(6):
                        issue_piece(g, p)
                    g += 1
                next_group = g
                next_piece = 0

                # thr(0) from v0 (ACT Sign matches reference sign(0)=0)
                sgn0 = small.tile([128, COLS], F32, name="sgn0", tag="sgn0")
                thr0 = small.tile([128, COLS], F32, name="thr0", tag="thr0")
                nc.scalar.sign(sgn0[:], v0t[:])
                nc.vector.scalar_tensor_tensor(
                    thr0[:], sgn0[:], -C0, nixp3[:, 0, :],
                    AluOpType.mult, AluOpType.add)
                thr_prev = thr0

                # ---- main loop ----
                zprev, zprev_off = z0t, 0
                for t in range(T):
                    # keep phase A >= LA steps ahead
                    want = min(n_groups, (t + LA) // G + 1)
                    budget = PIECES_PER_STEP
                    while next_group < want and budget > 0:
                        issue_piece(next_group, next_piece)
                        next_piece += 1
                        budget -= 1
                        if next_piece == 6:
                            next_piece = 0
                            next_group += 1
                    if t % CH == 0:
                        load_chunk(t // CH + 2)
                    ps = psumB.tile([128, COLS], F32, name="psB", tag="psB")
                    for j in range(2):
                        for i in range(2):
                            nc.tensor.matmul(
                                ps[:, j * BC:(j + 1) * BC],
                                wrec_t[i][j][:],
                                zprev[:, zprev_off + i * BC:
                                      zprev_off + (i + 1) * BC],
                                start=(i == 0), stop=(i == 1),
                                skip_group_check=True)

                    slot = (t % RING) * COLS
                    z_ap = zring[:, slot:slot + COLS]
                    v_ap = vring[:, slot:slot + COLS]
                    ve = nc.vector
                    # critical op: z(t) = (i_rec > thr(t-1)), output in RDT
                    ve.tensor_tensor(z_ap, ps[:], thr_prev[:], AluOpType.is_gt)
                    # bookkeeping (overlaps next step's matmuls):
                    # v(t) = ps + 0.5 - thr(t-1)
                    ve.scalar_tensor_tensor(
                        v_ap, ps[:], 0.5, thr_prev[:],
                        AluOpType.add, AluOpType.subtract)
                    if t < T - 1:
                        nslot = (t + 1) % NR
                        thr_n = small.tile([128, COLS], F32, name="thr",
                                           tag=f"thr{t % 2}")
                        s0 = small.tile([128, COLS], F32, name="s0",
                                        tag=f"s0{t % 2}")
                        # s0 = copysign(|C0|, v)
                        ve.tensor_scalar(
                            s0[:].bitcast(I32), v_ap.bitcast(I32),
                            SIGNMASK, ABS_C0_BITS,
                            AluOpType.bitwise_and, AluOpType.bitwise_xor)
                        # thr(t) = s0 + (0.5 - i_x(t+1))
                        ve.tensor_add(thr_n[:], s0[:], nixp3[:, nslot, :])
                        thr_prev = thr_n

                    zprev, zprev_off = zring, slot
                    if (t + 1) % DRAIN == 0 or t == T - 1:
                        d0 = (t // DRAIN) * DRAIN
                        nsteps = t + 1 - d0
                        rs = (d0 % RING) * COLS
                        nc.sync.dma_start(
                            out=zout_d[:, d0 * COLS:(t + 1) * COLS],
                            in_=zring[:, rs:rs + nsteps * COLS])
                        nc.sync.dma_start(
                            out=vout_d[:, d0 * COLS:(t + 1) * COLS],
                            in_=vring[:, rs:rs + nsteps * COLS])
    nc.compile()
    return nc


def _get_program(T):
    if T not in _program_cache:
        _program_cache[T] = build_program(T)
    return _program_cache[T]


def _shard_host(x, z0, v0, w_in, w_rec):
    """Build per-core input maps (host-side layout transforms only)."""
    T = x.shape[0]
    in_maps = []
    for c in range(N_CORES):
        sl = slice(c * BC, (c + 1) * BC)
        xc = np.ascontiguousarray(
            x[:, sl, :].transpose(2, 0, 1).reshape(N, T * BC).astype(np.float32))
        z0c = np.ascontiguousarray(
            z0[sl, :].T.reshape(2, 128, BC).transpose(1, 0, 2).reshape(128, COLS)
            .astype(np.float16))
        v0c = np.ascontiguousarray(
            v0[sl, :].T.reshape(2, 128, BC).transpose(1, 0, 2).reshape(128, COLS)
            .astype(np.float32))
        in_maps.append({
            "xt": xc,
            "w_in": np.ascontiguousarray(w_in.astype(np.float32)),
            "w_rec": np.ascontiguousarray(w_rec.astype(np.float16)),
            "z0t": z0c,
            "v0t": v0c,
        })
    return in_maps


def _unshard(res_list, T):
    zs = np.empty((T, B_FULL, N), np.float32)
    vs = np.empty((T, B_FULL, N), np.float32)
    for c, out in enumerate(res_list):
        sl = slice(c * BC, (c + 1) * BC)
        # [128, T*32] -> [p, t, j, b] -> [t, b, j*128+p]; z arrives fp16
        z = (np.asarray(out["z_out"]).astype(np.float32)
             .reshape(128, T, 2, BC).transpose(1, 3, 2, 0))
        v = np.asarray(out["v_out"]).reshape(128, T, 2, BC).transpose(1, 3, 2, 0)
        zs[:, sl, :] = z.reshape(T, BC, N)
        vs[:, sl, :] = v.reshape(T, BC, N)
    return zs, vs


def _make_runner(nc, n_cores):
    """jit(shard_map(bass_exec)) runner with device-resident inputs."""
    import jax
    from jax.sharding import Mesh, PartitionSpec, NamedSharding
    from jax.experimental.shard_map import shard_map
    from concourse import bass2jax as b2j

    b2j.install_neuronx_cc_hook()
    partition_name = nc.partition_id_tensor.name if nc.partition_id_tensor else None
    in_names, out_names, out_avals, zero_outs = [], [], [], []
    for alloc in nc.m.functions[0].allocations:
        if not isinstance(alloc, mybir.MemoryLocationSet):
            continue
        name = alloc.memorylocations[0].name
        if alloc.kind == "ExternalInput":
            if name != partition_name:
                in_names.append(name)
        elif alloc.kind == "ExternalOutput":
            shape = tuple(alloc.tensor_shape)
            dtype = mybir.dt.np(alloc.dtype)
            out_names.append(name)
            out_avals.append(jax.core.ShapedArray(shape, dtype))
            zero_outs.append(np.zeros(shape, dtype))
    n_params = len(in_names)
    n_outs = len(out_avals)
    in_names_all = in_names + out_names
    if partition_name is not None:
        in_names_all.append(partition_name)

    def _body(*args):
        operands = list(args)
        if partition_name is not None:
            operands.append(b2j.partition_id_tensor())
        return tuple(b2j._bass_exec_p.bind(
            *operands, out_avals=tuple(out_avals), in_names=tuple(in_names_all),
            out_names=tuple(out_names), lowering_input_output_aliases=(),
            sim_require_finite=True, sim_require_nnan=True, nc=nc))

    devices = jax.devices()[:n_cores]
    mesh = Mesh(np.asarray(devices), ("core",))
    donate = tuple(range(n_params, n_params + n_outs))
    sharded = jax.jit(
        shard_map(_body, mesh=mesh,
                  in_specs=(PartitionSpec("core"),) * (n_params + n_outs),
                  out_specs=(PartitionSpec("core"),) * n_outs,
                  check_rep=False),
        donate_argnums=donate, keep_unused=True)
    sh = NamedSharding(mesh, PartitionSpec("core"))
    return sharded, sh, in_names, out_names, out_avals, zero_outs


def _run_timed(nc, in_maps, repeats=8):
    """Best wall time of one dispatched call (includes the axon floor)."""
    import time
    import jax

    n_cores = len(in_maps)
    sharded, sh, in_names, out_names, out_avals, zero_outs = \
        _make_runner(nc, n_cores)
    concat_in = [np.concatenate([np.asarray(m[n]) for m in in_maps], axis=0)
                 for n in in_names]
    din = [jax.device_put(a, sh) for a in concat_in]
    out_arrs = [jax.device_put(
        np.zeros((n_cores * z.shape[0], *z.shape[1:]), z.dtype), sh)
        for z in zero_outs]
    jax.block_until_ready(out_arrs)
    jax.block_until_ready(din)
    best = None
    for _ in range(max(1, repeats)):
        t0 = time.perf_counter()
        out_arrs = sharded(*din, *out_arrs)
        jax.block_until_ready(out_arrs)
        dt = time.perf_counter() - t0
        best = dt if best is None else min(best, dt)
    results = [{name: np.asarray(out_arrs[i]).reshape(n_cores, *out_avals[i].shape)[c]
                for i, name in enumerate(out_names)}
               for c in range(n_cores)]
    return results, int(best * 1e9)


def kernel(x, z0, v0, t0, w_in, w_rec):
    T = x.shape[0]
    nc = _get_program(T)
    in_maps = _shard_host(np.asarray(x), np.asarray(z0), np.asarray(v0),
                          np.asarray(w_in), np.asarray(w_rec))
    res = run_bass_kernel_spmd(nc, in_maps, list(range(N_CORES)), trace=False)
    return _unshard(res.results, T)
